# revision 1
# baseline (speedup 1.0000x reference)
"""Trainium2 Bass kernel for nn_LDRFat (3-layer MLP forward).

reference: logits = relu((x @ W) @ fc_w.T + fc_b) @ logits_w.T + logits_b

Key algebraic optimization: (x @ W) @ fc_w.T == x @ (W @ fc_w.T).
Precomputing Wfc = W @ fc_w.T ([3072,512], 9.7 GFLOP) collapses the
dominant 309 GFLOP x@W matmul into a 51.5 GFLOP x@Wfc.

Sharding: data-parallel over batch for the main pass (2048 rows/core).
The Wfc precompute is sharded over W's rows (each core gets its own
Wshard input, 384 rows) and the 8 shards are combined with an AllGather
collective. Set KERNEL_V1=1 for the no-collective fallback (every core
redundantly computes all of Wfc from the full W input).

Matmuls run as float32r (FP22 multiply, fp32 accumulate) = full PE rate.
Transposes (PE transpose mode, plain fp32) are exact. Transposes are
batched into dense runs separate from matmul runs: PE transpose-mode
doesn't count as busy for the HAM clock gate, so interleaving T/MM kept
the PE at 1.2 GHz (measured 6x slowdown on phase B).
"""

import os
import numpy as np

import concourse.bass as bass
import concourse.mybir as mybir
import concourse.tile as tile
from concourse import bacc
from concourse.bass import MemorySpace, ts, ds
from concourse.bass_utils import run_bass_kernel_spmd
from concourse.masks import make_identity

B = 16384
N = 3072
FC = 512
CLS = 10
NCORES = 8
BS = B // NCORES   # 2048 rows per core
P = 128

KT = N // P        # 24 k-tiles
NT = N // P        # 24 n-tiles
FT = FC // P       # 4 f-tiles
MCHUNK = 512
NMC = BS // MCHUNK   # 4 m-chunks per core
MSUB = MCHUNK // P   # 4 sub-tiles per chunk
KSH = KT // NCORES   # 3 k-tiles per core in sharded precompute
WROWS = KSH * P      # 384 W-rows per core

F32 = mybir.dt.float32
F32R = mybir.dt.float32r

_CACHE = {}
LAST_RESULT = None


def _build_fcwT(nc, tc, ps_tp, fcw_d, identity, fcwT):
    """fc_wT[n, f] tiles via PE transposes (dense-batched)."""
    with tc.tile_pool(name="fcw_nat", bufs=2) as fcw_nat_pool:
        for ft in range(FT):
            fstrip = fcw_nat_pool.tile([P, N], F32, tag="fcwstrip")
            nc.sync.dma_start(fstrip, fcw_d[ts(ft, P), :])
            for nt in range(NT):
                pst = ps_tp.tile([P, P], F32, tag="tp")
                nc.tensor.transpose(pst, fstrip[:, ts(nt, P)], identity)
                nc.vector.tensor_copy(fcwT[:, nt, ts(ft, P)], pst)


def _wfc_shard_compute(nc, tc, ps_tp, ps_acc, w_src, fcwT, dst, nkt,
                       w_strip_pool, wTs_pool):
    """dst[:, lkt] = Wfc rows for k-tiles of w_src (nkt tiles)."""
    for lkt in range(nkt):
        wstrip = w_strip_pool.tile([P, N], F32, tag="wstrip")
        nc.sync.dma_start(wstrip, w_src[ts(lkt, P), :])
        wTs = wTs_pool.tile([P, NT, P], F32R, tag="wTs")
        for nt in range(NT):
            pst = ps_tp.tile([P, P], F32, tag="tp")
            nc.tensor.transpose(pst, wstrip[:, ts(nt, P)], identity_g[0])
            nc.vector.tensor_copy(wTs[:, nt], pst)
        acc = ps_acc.tile([P, FC], F32, tag="acc")
        for nt in range(NT):
            nc.tensor.matmul(
                acc, wTs[:, nt], fcwT[:, nt],
                start=(nt == 0), stop=(nt == NT - 1),
            )
        nc.vector.tensor_copy(dst[:, lkt], acc)


identity_g = [None]


def build_kernel(phase=None):
    phase = phase or os.environ.get("KERNEL_PHASE", "both")
    v1 = bool(int(os.environ.get("KERNEL_V1", "1")))
    repeat = int(os.environ.get("KERNEL_REPEAT", "1"))

    nc = bacc.Bacc(
        "TRN2",
        target_bir_lowering=False,
        debug=False,
        enable_asserts=False,
        num_devices=NCORES,
    )
    x_d = nc.dram_tensor("x", [BS, N], F32, kind="ExternalInput").ap()
    if v1:
        w_d = nc.dram_tensor("W", [N, N], F32, kind="ExternalInput").ap()
    else:
        wsh_d = nc.dram_tensor("Wshard", [WROWS, N], F32, kind="ExternalInput").ap()
    fcw_d = nc.dram_tensor("fc_w", [FC, N], F32, kind="ExternalInput").ap()
    fcb_d = nc.dram_tensor("fc_b", [FC], F32, kind="ExternalInput").ap()
    lgw_d = nc.dram_tensor("logits_w", [CLS, FC], F32, kind="ExternalInput").ap()
    lgb_d = nc.dram_tensor("logits_b", [CLS], F32, kind="ExternalInput").ap()
    out_d = nc.dram_tensor("out", [BS, CLS], F32, kind="ExternalOutput").ap()

    with tile.TileContext(nc) as tc:
        with (
            tc.tile_pool(name="consts", bufs=1) as consts,
            tc.tile_pool(name="wfc", bufs=1) as wfc_pool,
            tc.tile_pool(name="ps_acc", bufs=4, space=MemorySpace.PSUM) as ps_acc,
            tc.tile_pool(name="ps_tp", bufs=3, space=MemorySpace.PSUM) as ps_tp,
            tc.tile_pool(name="ps_lg", bufs=1, space=MemorySpace.PSUM) as ps_lg,
        ):
            identity = consts.tile([P, P], F32)
            make_identity(nc, identity)
            identity_g[0] = identity

            fcb_sb = consts.tile([P, FT], F32)
            nc.sync.dma_start(fcb_sb, fcb_d.rearrange("(t p) -> p t", p=P))

            lgw_sb = consts.tile([CLS, FC], F32)
            nc.sync.dma_start(lgw_sb, lgw_d)
            lgb_stage = consts.tile([1, CLS], F32)
            nc.sync.dma_start(lgb_stage, lgb_d.rearrange("(a c) -> a c", a=1))
            lgb_sb = consts.tile([1, CLS], F32R)
            nc.vector.tensor_copy(lgb_sb, lgb_stage)
            ones_stage = consts.tile([1, P], F32)
            nc.gpsimd.memset(ones_stage, 1.0)
            ones_sb = consts.tile([1, P], F32R)
            nc.vector.tensor_copy(ones_sb, ones_stage)

            lgwT_sb = consts.tile([P, FT, CLS], F32R)
            for ft in range(FT):
                pst = ps_tp.tile([P, P], F32, tag="tp")
                nc.tensor.transpose(
                    pst[:, :CLS], lgw_sb[:, ts(ft, P)], identity[:CLS, :CLS]
                )
                nc.vector.tensor_copy(lgwT_sb[:, ft], pst[:, :CLS])

            # Wfc[k, f] = sum_n W[k, n] fc_w[f, n]; resident all of phase B
            wfc_sb = wfc_pool.tile([P, KT, FC], F32R)

            # ---------------- Phase A ----------------
            if phase in ("both", "a") and not v1:
                # sharded precompute + AllGather
                with (
                    tc.tile_pool(name="fcwT_p", bufs=1) as fcwT_pool,
                    tc.tile_pool(name="w_strip", bufs=2) as w_strip_pool,
                    tc.tile_pool(name="wTs_p", bufs=2) as wTs_pool,
                    tc.tile_pool(name="wfc_stage", bufs=1) as wfc_stage_pool,
                    tc.tile_pool(name="cc_dram", bufs=1, space=MemorySpace.DRAM) as ccd,
                ):
                    fcwT = fcwT_pool.tile([P, NT, FC], F32R)
                    _build_fcwT(nc, tc, ps_tp, fcw_d, identity, fcwT)

                    wfc_stage = wfc_stage_pool.tile([P, KSH, FC], F32R)
                    _wfc_shard_compute(nc, tc, ps_tp, ps_acc, wsh_d, fcwT,
                                       wfc_stage, KSH, w_strip_pool, wTs_pool)

                    gin = ccd.tile([P, KSH * FC], F32R)
                    nc.sync.dma_start(
                        gin, wfc_stage.rearrange("p a b -> p (a b)")
                    )
                    gout = ccd.tile([NCORES * P, KSH * FC], F32R)
                    nc.gpsimd.collective_compute(
                        "AllGather",
                        mybir.AluOpType.bypass,
                        replica_groups=[list(range(NCORES))],
                        ins=[gin.opt()],
                        outs=[gout.opt()],
                    )
                    # gout rows = (core c, partition p); free j = (lkt, f)
                    nc.sync.dma_start(
                        wfc_sb.rearrange("p (c l) f -> p c (l f)", c=NCORES),
                        gout.rearrange("(c p) j -> p c j", p=P),
                    )

            if phase in ("both", "a") and v1:
                with (
                    tc.tile_pool(name="fcwT_p", bufs=1) as fcwT_pool,
                    tc.tile_pool(name="w_strip", bufs=2) as w_strip_pool,
                    tc.tile_pool(name="wTs_p", bufs=2) as wTs_pool,
                ):
                    fcwT = fcwT_pool.tile([P, NT, FC], F32R)
                    _build_fcwT(nc, tc, ps_tp, fcw_d, identity, fcwT)
                    for _arep in range(int(os.environ.get("KERNEL_REPEAT_A", "1"))):
                        _wfc_shard_compute(nc, tc, ps_tp, ps_acc, w_d, fcwT,
                                           wfc_sb, KT, w_strip_pool, wTs_pool)

            if phase == "b":
                nc.gpsimd.memset(wfc_sb.bitcast(F32), 0.0)
            if phase == "a":
                with tc.tile_pool(name="dbg_dram", bufs=1, space=MemorySpace.DRAM) as dp:
                    wfc_dump = dp.tile([P, KT * FC], F32)
                    nc.sync.dma_start(
                        wfc_dump, wfc_sb.bitcast(F32).rearrange("p a b -> p (a b)")
                    )
                    dump = consts.tile([P, CLS], F32)
                    nc.vector.tensor_copy(dump, wfc_sb[:, 0, :CLS].bitcast(F32))
                    nc.sync.dma_start(out_d[:P, :], dump)

            # ---------------- Phase B ----------------
            if phase in ("both", "b"):
                with (
                    tc.tile_pool(name="x_nat", bufs=5) as x_nat_pool,
                    tc.tile_pool(name="xT", bufs=1) as xT_pool,
                    tc.tile_pool(name="yT", bufs=2) as yT_pool,
                    tc.tile_pool(name="out_sb", bufs=3) as out_pool,
                ):
                    for rep in range(repeat):
                        for mc in range(NMC):
                            xs = []
                            for msub in range(MSUB):
                                xn = x_nat_pool.tile([P, N], F32, tag="xnat")
                                nc.sync.dma_start(
                                    xn, x_d[ds(mc * MCHUNK + msub * P, P), :]
                                )
                                xs.append(xn)

                            # dense transpose run for the whole chunk
                            xTs = xT_pool.tile([P, KT, MCHUNK], F32R, tag="xTs")
                            for kt in range(KT):
                                for msub in range(MSUB):
                                    pst = ps_tp.tile([P, P], F32, tag="tp")
                                    nc.tensor.transpose(
                                        pst, xs[msub][:, ts(kt, P)], identity
                                    )
                                    nc.vector.tensor_copy(
                                        xTs[:, kt, ts(msub, P)], pst
                                    )

                            # dense matmul run
                            h2 = [
                                ps_acc.tile(
                                    [P, MCHUNK], F32, tag="acc",
                                    name=f"h2_{rep}_{mc}_{ft}",
                                )
                                for ft in range(FT)
                            ]
                            for kt in range(KT):
                                for ft in range(FT):
                                    nc.tensor.matmul(
                                        h2[ft],
                                        wfc_sb[:, kt, ts(ft, P)],
                                        xTs[:, kt],
                                        start=(kt == 0),
                                        stop=(kt == KT - 1),
                                    )

                            # relu(h2 + fc_b), per-partition bias on ACT
                            yT = yT_pool.tile([P, FT, MCHUNK], F32R, tag="yT")
                            for ft in range(FT):
                                nc.scalar.activation(
                                    yT[:, ft],
                                    h2[ft],
                                    mybir.ActivationFunctionType.Relu,
                                    bias=fcb_sb[:, ds(ft, 1)],
                                )

                            # logits + bias (K=1 ones x logits_b matmul)
                            for msub in range(MSUB):
                                plg = ps_lg.tile([P, CLS], F32, tag="lg")
                                for ft in range(FT):
                                    nc.tensor.matmul(
                                        plg,
                                        yT[:, ft, ts(msub, P)],
                                        lgwT_sb[:, ft],
                                        start=(ft == 0),
                                        stop=False,
                                    )
                                nc.tensor.matmul(
                                    plg, ones_sb, lgb_sb, start=False, stop=True
                                )
                                osb = out_pool.tile([P, CLS], F32, tag="osb")
                                nc.vector.tensor_copy(osb, plg)
                                nc.sync.dma_start(
                                    out_d[ds(mc * MCHUNK + msub * P, P), :], osb
                                )

    nc.compile()
    return nc


def kernel(**inputs) -> np.ndarray:
    global LAST_RESULT
    if "nc" not in _CACHE:
        _CACHE["nc"] = build_kernel()
    nc = _CACHE["nc"]
    v1 = bool(int(os.environ.get("KERNEL_V1", "1")))

    x = np.ascontiguousarray(inputs["x"], dtype=np.float32)
    W = np.ascontiguousarray(inputs["W"], dtype=np.float32)
    fc_w = np.ascontiguousarray(inputs["fc_w"], dtype=np.float32)
    fc_b = np.ascontiguousarray(inputs["fc_b"], dtype=np.float32)
    lgw = np.ascontiguousarray(inputs["logits_w"], dtype=np.float32)
    lgb = np.ascontiguousarray(inputs["logits_b"], dtype=np.float32)

    in_maps = []
    for i in range(NCORES):
        m = {
            "x": x[i * BS : (i + 1) * BS],
            "fc_w": fc_w,
            "fc_b": fc_b,
            "logits_w": lgw,
            "logits_b": lgb,
        }
        if v1:
            m["W"] = W
        else:
            m["Wshard"] = np.ascontiguousarray(W[i * WROWS : (i + 1) * WROWS])
        in_maps.append(m)

    res = run_bass_kernel_spmd(
        nc,
        in_maps,
        core_ids=list(range(NCORES)),
        trace=bool(int(os.environ.get("KERNEL_TRACE", "0"))),
    )
    LAST_RESULT = res
    out = np.concatenate([r_["out"] for r_ in res.results], axis=0)
    return out



# revision 2
# speedup vs baseline: 1.3433x; 1.3433x over previous
"""Trainium2 Bass kernel for nn_LDRFat (3-layer MLP forward).

reference: logits = relu((x @ W) @ fc_w.T + fc_b) @ logits_w.T + logits_b

Algebraic optimization: (x @ W) @ fc_w.T == x @ (W @ fc_w.T).
Precomputing Wfc = W @ fc_w.T ([3072,512]) collapses the dominant
309 GFLOP x@W matmul into a 51.5 GFLOP x@Wfc.

Layout strategy: ALL transposes happen on the host (numpy) inside
kernel() — the device graph is a pure matmul stream, which also keeps
the PE HAM clock gate warm (transpose-mode ops don't count as busy and
previously kept the PE at 1.2 GHz).

  - x is fed as xT [N, BS] per core (batch-sharded, 2048 rows/core)
  - W is fed as WT k-shard [N, 384] per core (phase A sharded 8-way)
  - fc_w fed as fcwT [N, FC]; logits_w fed as lgwT [FC, CLS]

Phase A: each core computes its 384 rows of Wfc = W @ fc_w.T
(72 accumulating matmuls), then a bf16 AllGather replicates the full
Wfc. Phase B: h^T tiles [f, m] = Wfc^T-contracted matmuls over n,
ReLU+bias on the scalar engine, then a 4-matmul logits head with
lgwT stationary producing out^T [10, m]; logits bias is applied as a
per-partition bias in the scalar-engine PSUM->SBUF copy. The final
output transpose ([10, BS] -> [BS, 10]) happens on the host.

All matmul operands are bf16 (fp32 PSUM accumulation): same PE rate as
fp32r but half the DMA/SBUF/collective bytes. Measured rel err ~1e-3
vs the 2e-2 gate.

Set KERNEL_V1=1 for the no-collective fallback (every core redundantly
computes all of Wfc from the full WT input).
"""

import os
import numpy as np
import ml_dtypes

import concourse.bass as bass
import concourse.mybir as mybir
import concourse.tile as tile
from concourse import bacc
from concourse.bass import MemorySpace, ts, ds
from concourse.bass_utils import run_bass_kernel_spmd

B = 16384
N = 3072
FC = 512
CLS = 10
NCORES = 8
BS = B // NCORES     # 2048 batch rows per core
P = 128

KT = N // P          # 24 contraction tiles
FT = FC // P         # 4 f-tiles
MCHUNK = 512
NMC = BS // MCHUNK   # 4 m-chunks per core
KSH = KT // NCORES   # 3 k-tiles per core in sharded precompute
WROWS = KSH * P      # 384 W-rows per core

F32 = mybir.dt.float32
BF16 = mybir.dt.bfloat16
BF = ml_dtypes.bfloat16

_CACHE = {}
LAST_RESULT = None


def build_kernel():
    v1 = bool(int(os.environ.get("KERNEL_V1", "0")))

    nc = bacc.Bacc(
        "TRN2",
        target_bir_lowering=False,
        debug=False,
        enable_asserts=False,
        num_devices=NCORES,
    )
    xT_d = nc.dram_tensor("xT", [N, BS], BF16, kind="ExternalInput").ap()
    wrows = N if v1 else WROWS
    wT_d = nc.dram_tensor("WT", [N, wrows], BF16, kind="ExternalInput").ap()
    fwT_d = nc.dram_tensor("fcwT", [N, FC], BF16, kind="ExternalInput").ap()
    fcb_d = nc.dram_tensor("fc_b", [FC], F32, kind="ExternalInput").ap()
    lwT_d = nc.dram_tensor("lgwT", [FC, CLS], BF16, kind="ExternalInput").ap()
    lgb_d = nc.dram_tensor("lgb", [CLS, 1], F32, kind="ExternalInput").ap()
    outT_d = nc.dram_tensor("outT", [CLS, BS], F32, kind="ExternalOutput").ap()

    with tile.TileContext(nc) as tc:
        with (
            tc.tile_pool(name="consts", bufs=1) as consts,
            tc.tile_pool(name="big", bufs=1) as big,
            tc.tile_pool(name="stage", bufs=1) as stage_pool,
            tc.tile_pool(name="yT_p", bufs=2) as yT_pool,
            tc.tile_pool(name="ps_a", bufs=2, space=MemorySpace.PSUM) as ps_a,
            tc.tile_pool(name="ps_b", bufs=4, space=MemorySpace.PSUM) as ps_b,
            tc.tile_pool(name="ps_lg", bufs=2, space=MemorySpace.PSUM) as ps_lg,
            tc.tile_pool(name="cc_dram", bufs=1, space=MemorySpace.DRAM) as ccd,
        ):
            fcb_sb = consts.tile([P, FT], F32)
            nc.sync.dma_start(fcb_sb, fcb_d.rearrange("(t p) -> p t", p=P))
            lgw_sb = consts.tile([P, FT, CLS], BF16)
            nc.sync.dma_start(lgw_sb, lwT_d.rearrange("(t p) c -> p t c", p=P))
            lgb_sb = consts.tile([CLS, 1], F32)
            nc.sync.dma_start(lgb_sb, lgb_d)

            # phase A inputs first (phase A compute gates the collective),
            # then the x strips (consumed later, in phase B).
            fwT_sb = big.tile([P, KT, FC], BF16)
            for nt in range(KT):
                nc.sync.dma_start(fwT_sb[:, nt], fwT_d[ts(nt, P), :])
            wT_sb = big.tile([P, KT, wrows], BF16)
            for nt in range(KT):
                nc.sync.dma_start(wT_sb[:, nt], wT_d[ts(nt, P), :])
            x_sb = big.tile([P, KT, BS], BF16)
            for nt in range(KT):
                nc.sync.dma_start(x_sb[:, nt], xT_d[ts(nt, P), :])

            # ---------------- Phase A: Wfc rows ----------------
            wfc_sb = big.tile([P, KT, FC], BF16)
            nkt = KT if v1 else KSH
            wfc_stage = stage_pool.tile([P, nkt, FC], BF16)
            for lkt in range(nkt):
                acc = ps_a.tile([P, FC], F32, tag="acc")
                for nt in range(KT):
                    nc.tensor.matmul(
                        acc,
                        wT_sb[:, nt, ts(lkt, P)],
                        fwT_sb[:, nt],
                        start=(nt == 0),
                        stop=(nt == KT - 1),
                    )
                nc.vector.tensor_copy(wfc_stage[:, lkt], acc)

            if v1:
                nc.vector.tensor_copy(wfc_sb, wfc_stage)
            else:
                gin = ccd.tile([P, KSH * FC], BF16)
                nc.sync.dma_start(gin, wfc_stage.rearrange("p a b -> p (a b)"))
                gout = ccd.tile([NCORES * P, KSH * FC], BF16)
                nc.gpsimd.collective_compute(
                    "AllGather",
                    mybir.AluOpType.bypass,
                    replica_groups=[list(range(NCORES))],
                    ins=[gin.opt()],
                    outs=[gout.opt()],
                )
                # gout rows = (core c, partition p); free j = (lkt, f)
                nc.sync.dma_start(
                    wfc_sb.rearrange("p (c l) f -> p c (l f)", c=NCORES),
                    gout.rearrange("(c p) j -> p c j", p=P),
                )

            # ---------------- Phase B ----------------
            for mc in range(NMC):
                h2 = [
                    ps_b.tile([P, MCHUNK], F32, tag="acc", name=f"h2_{mc}_{ft}")
                    for ft in range(FT)
                ]
                for kt in range(KT):
                    for ft in range(FT):
                        nc.tensor.matmul(
                            h2[ft],
                            wfc_sb[:, kt, ts(ft, P)],
                            x_sb[:, kt, ts(mc, MCHUNK)],
                            start=(kt == 0),
                            stop=(kt == KT - 1),
                        )

                # relu(h2 + fc_b) on the scalar engine, cast to bf16
                yT = yT_pool.tile([P, FT, MCHUNK], BF16, tag="yT")
                for ft in range(FT):
                    nc.scalar.activation(
                        yT[:, ft],
                        h2[ft],
                        mybir.ActivationFunctionType.Relu,
                        bias=fcb_sb[:, ds(ft, 1)],
                    )

                # logits head: lgwT stationary, yT moving -> outT [CLS, m]
                plg = ps_lg.tile([CLS, MCHUNK], F32, tag="lg")
                for ft in range(FT):
                    nc.tensor.matmul(
                        plg,
                        lgw_sb[:, ft],
                        yT[:, ft],
                        start=(ft == 0),
                        stop=(ft == FT - 1),
                    )
                # + logits_b as per-partition bias during PSUM->SBUF copy
                osb = yT_pool.tile([CLS, MCHUNK], F32, tag="osb")
                nc.scalar.activation(
                    osb,
                    plg,
                    mybir.ActivationFunctionType.Identity,
                    bias=lgb_sb[:, ds(0, 1)],
                )
                nc.sync.dma_start(outT_d[:, ts(mc, MCHUNK)], osb)

    nc.compile()
    return nc


def kernel(**inputs) -> np.ndarray:
    global LAST_RESULT
    key = "nc_v1" if int(os.environ.get("KERNEL_V1", "0")) else "nc"
    if key not in _CACHE:
        _CACHE[key] = build_kernel()
    nc = _CACHE[key]
    v1 = bool(int(os.environ.get("KERNEL_V1", "0")))

    x = np.asarray(inputs["x"], dtype=np.float32)
    W = np.asarray(inputs["W"], dtype=np.float32)
    fc_w = np.asarray(inputs["fc_w"], dtype=np.float32)
    fc_b = np.asarray(inputs["fc_b"], dtype=np.float32)
    lgw = np.asarray(inputs["logits_w"], dtype=np.float32)
    lgb = np.asarray(inputs["logits_b"], dtype=np.float32)

    fcwT = np.ascontiguousarray(fc_w.T).astype(BF)        # [N, FC]
    lgwT = np.ascontiguousarray(lgw.T).astype(BF)         # [FC, CLS]
    lgb_col = np.ascontiguousarray(lgb.reshape(CLS, 1))   # [CLS, 1]
    if v1:
        wT_full = np.ascontiguousarray(W.T).astype(BF)    # [N, N]

    in_maps = []
    for i in range(NCORES):
        xT = np.ascontiguousarray(x[i * BS : (i + 1) * BS].T).astype(BF)
        if v1:
            wT = wT_full
        else:
            wT = np.ascontiguousarray(W[i * WROWS : (i + 1) * WROWS].T).astype(BF)
        in_maps.append(
            {
                "xT": xT,
                "WT": wT,
                "fcwT": fcwT,
                "fc_b": fc_b,
                "lgwT": lgwT,
                "lgb": lgb_col,
            }
        )

    res = run_bass_kernel_spmd(
        nc,
        in_maps,
        core_ids=list(range(NCORES)),
        trace=bool(int(os.environ.get("KERNEL_TRACE", "0"))),
    )
    LAST_RESULT = res
    out = np.concatenate(
        [np.ascontiguousarray(r_["outT"].T) for r_ in res.results], axis=0
    )
    return out


# revision 5
# speedup vs baseline: 1.4560x; 1.0839x over previous
"""Trainium2 Bass kernel for nn_LDRFat (3-layer MLP forward).

reference: logits = relu((x @ W) @ fc_w.T + fc_b) @ logits_w.T + logits_b

Algebraic optimization: (x @ W) @ fc_w.T == x @ (W @ fc_w.T).
Precomputing Wfc = W @ fc_w.T ([3072,512]) collapses the dominant
309 GFLOP x@W matmul into a 51.5 GFLOP x@Wfc.

Layout strategy: ALL transposes happen on the host (numpy) inside
kernel() — the device graph is a pure matmul stream, which also keeps
the PE HAM clock gate warm (transpose-mode ops don't count as busy and
previously kept the PE at 1.2 GHz).

  - x is fed as xT [N, BS] per core (batch-sharded, 2048 rows/core)
  - W is fed as WT k-shard [N, 384] per core (phase A sharded 8-way)
  - fc_w fed as fcwT [N, FC]; logits_w fed as lgwT [FC, CLS]

Phase A: each core computes its 384 rows of Wfc = W @ fc_w.T
(72 accumulating matmuls), then a bf16 AllGather replicates the full
Wfc. Phase B: h^T tiles [f, m] = Wfc^T-contracted matmuls over n,
ReLU+bias on the scalar engine, then a 4-matmul logits head with
lgwT stationary producing out^T [10, m]; logits bias is applied as a
per-partition bias in the scalar-engine PSUM->SBUF copy. The final
output transpose ([10, BS] -> [BS, 10]) happens on the host.

All matmul operands are bf16 (fp32 PSUM accumulation): same PE rate as
fp32r but half the DMA/SBUF/collective bytes. Measured rel err ~1e-3
vs the 2e-2 gate.

Set KERNEL_V1=1 for the no-collective fallback (every core redundantly
computes all of Wfc from the full WT input).
"""

import os
import numpy as np
import ml_dtypes

import concourse.bass as bass
import concourse.mybir as mybir
import concourse.tile as tile
from concourse import bacc
from concourse.bass import MemorySpace, ts, ds
from concourse.bass_utils import run_bass_kernel_spmd

B = 16384
N = 3072
FC = 512
CLS = 10
NCORES = 8
BS = B // NCORES     # 2048 batch rows per core
P = 128

KT = N // P          # 24 contraction tiles
FT = FC // P         # 4 f-tiles
MCHUNK = 512
NMC = BS // MCHUNK   # 4 m-chunks per core
KSH = KT // NCORES   # 3 k-tiles per core in sharded precompute
WROWS = KSH * P      # 384 W-rows per core

F32 = mybir.dt.float32
BF16 = mybir.dt.bfloat16
BF = ml_dtypes.bfloat16

_CACHE = {}
LAST_RESULT = None


def build_kernel():
    v1 = bool(int(os.environ.get("KERNEL_V1", "0")))

    nc = bacc.Bacc(
        "TRN2",
        target_bir_lowering=False,
        debug=False,
        enable_asserts=False,
        num_devices=NCORES,
    )
    xT_d = nc.dram_tensor("xT", [N, BS], BF16, kind="ExternalInput").ap()
    wrows = N if v1 else WROWS
    wT_d = nc.dram_tensor("WT", [N, wrows], BF16, kind="ExternalInput").ap()
    fwT_d = nc.dram_tensor("fcwT", [N, FC], BF16, kind="ExternalInput").ap()
    fcb_d = nc.dram_tensor("fc_b", [FC], F32, kind="ExternalInput").ap()
    lwT_d = nc.dram_tensor("lgwT", [FC, CLS], BF16, kind="ExternalInput").ap()
    lgb_d = nc.dram_tensor("lgb", [CLS, 1], F32, kind="ExternalInput").ap()
    outT_d = nc.dram_tensor("outT", [CLS, BS], F32, kind="ExternalOutput").ap()

    with tile.TileContext(nc) as tc:
        with (
            tc.tile_pool(name="consts", bufs=1) as consts,
            tc.tile_pool(name="big", bufs=1) as big,
            tc.tile_pool(name="stage", bufs=1) as stage_pool,
            tc.tile_pool(name="yT_p", bufs=2) as yT_pool,
            tc.tile_pool(name="ps_a", bufs=2, space=MemorySpace.PSUM) as ps_a,
            tc.tile_pool(name="ps_b", bufs=4, space=MemorySpace.PSUM) as ps_b,
            tc.tile_pool(name="ps_lg", bufs=2, space=MemorySpace.PSUM) as ps_lg,
            tc.tile_pool(name="cc_dram", bufs=1, space=MemorySpace.DRAM) as ccd,
        ):
            fcb_sb = consts.tile([P, FT], F32)
            nc.sync.dma_start(fcb_sb, fcb_d.rearrange("(t p) -> p t", p=P))
            lgw_sb = consts.tile([P, FT, CLS], BF16)
            nc.sync.dma_start(lgw_sb, lwT_d.rearrange("(t p) c -> p t c", p=P))
            lgb_sb = consts.tile([CLS, 1], F32)
            nc.sync.dma_start(lgb_sb, lgb_d)

            # phase A inputs first (phase A compute gates the collective),
            # interleaved per-strip so strip 0 of both lands earliest.
            fwT_sb = big.tile([P, KT, FC], BF16)
            wT_sb = big.tile([P, KT, wrows], BF16)
            for nt in range(KT):
                nc.sync.dma_start(fwT_sb[:, nt], fwT_d[ts(nt, P), :])
                nc.sync.dma_start(wT_sb[:, nt], wT_d[ts(nt, P), :])
            # x strips: 2 k-tiles per DMA (4KB lines). All on the sync
            # engine's rings; the collective's gin/gout DMAs go on the
            # activation engine's rings so they are never queued behind
            # these bulk loads.
            x_sb = big.tile([P, KT, BS], BF16)
            for g in range(KT // 2):
                nc.sync.dma_start(
                    x_sb[:, 2 * g : 2 * g + 2],
                    xT_d[ds(2 * g * P, 2 * P), :].rearrange(
                        "(t p) m -> p t m", p=P
                    ),
                )

            # ---------------- Phase A: Wfc rows ----------------
            wfc_sb = big.tile([P, KT, FC], BF16)
            nkt = KT if v1 else KSH
            wfc_stage = stage_pool.tile([P, nkt, FC], BF16)
            for lkt in range(nkt):
                acc = ps_a.tile([P, FC], F32, tag="acc")
                for nt in range(KT):
                    nc.tensor.matmul(
                        acc,
                        wT_sb[:, nt, ts(lkt, P)],
                        fwT_sb[:, nt],
                        start=(nt == 0),
                        stop=(nt == KT - 1),
                    )
                nc.vector.tensor_copy(wfc_stage[:, lkt], acc)

            if v1:
                nc.vector.tensor_copy(wfc_sb, wfc_stage)
            else:
                gin = ccd.tile([P, KSH * FC], BF16)
                nc.scalar.dma_start(gin, wfc_stage.rearrange("p a b -> p (a b)"))
                gout = ccd.tile([NCORES * P, KSH * FC], BF16)
                nc.gpsimd.collective_compute(
                    "AllGather",
                    mybir.AluOpType.bypass,
                    replica_groups=[list(range(NCORES))],
                    ins=[gin.opt()],
                    outs=[gout.opt()],
                )
                # gout rows = (core c, partition p); free j = (lkt, f)
                nc.scalar.dma_start(
                    wfc_sb.rearrange("p (c l) f -> p c (l f)", c=NCORES),
                    gout.rearrange("(c p) j -> p c j", p=P),
                )

            # ---------------- Phase B ----------------
            for mc in range(NMC):
                h2 = [
                    ps_b.tile([P, MCHUNK], F32, tag="acc", name=f"h2_{mc}_{ft}")
                    for ft in range(FT)
                ]
                for kt in range(KT):
                    for ft in range(FT):
                        nc.tensor.matmul(
                            h2[ft],
                            wfc_sb[:, kt, ts(ft, P)],
                            x_sb[:, kt, ts(mc, MCHUNK)],
                            start=(kt == 0),
                            stop=(kt == KT - 1),
                        )

                # relu(h2 + fc_b) on the scalar engine, cast to bf16
                yT = yT_pool.tile([P, FT, MCHUNK], BF16, tag="yT")
                for ft in range(FT):
                    nc.scalar.activation(
                        yT[:, ft],
                        h2[ft],
                        mybir.ActivationFunctionType.Relu,
                        bias=fcb_sb[:, ds(ft, 1)],
                    )

                # logits head: lgwT stationary, yT moving -> outT [CLS, m]
                plg = ps_lg.tile([CLS, MCHUNK], F32, tag="lg")
                for ft in range(FT):
                    nc.tensor.matmul(
                        plg,
                        lgw_sb[:, ft],
                        yT[:, ft],
                        start=(ft == 0),
                        stop=(ft == FT - 1),
                    )
                # + logits_b as per-partition bias during PSUM->SBUF copy
                osb = yT_pool.tile([CLS, MCHUNK], F32, tag="osb")
                nc.scalar.activation(
                    osb,
                    plg,
                    mybir.ActivationFunctionType.Identity,
                    bias=lgb_sb[:, ds(0, 1)],
                )
                nc.scalar.dma_start(outT_d[:, ts(mc, MCHUNK)], osb)

    nc.compile()
    return nc


def kernel(**inputs) -> np.ndarray:
    global LAST_RESULT
    key = "nc_v1" if int(os.environ.get("KERNEL_V1", "0")) else "nc"
    if key not in _CACHE:
        _CACHE[key] = build_kernel()
    nc = _CACHE[key]
    v1 = bool(int(os.environ.get("KERNEL_V1", "0")))

    x = np.asarray(inputs["x"], dtype=np.float32)
    W = np.asarray(inputs["W"], dtype=np.float32)
    fc_w = np.asarray(inputs["fc_w"], dtype=np.float32)
    fc_b = np.asarray(inputs["fc_b"], dtype=np.float32)
    lgw = np.asarray(inputs["logits_w"], dtype=np.float32)
    lgb = np.asarray(inputs["logits_b"], dtype=np.float32)

    fcwT = np.ascontiguousarray(fc_w.T).astype(BF)        # [N, FC]
    lgwT = np.ascontiguousarray(lgw.T).astype(BF)         # [FC, CLS]
    lgb_col = np.ascontiguousarray(lgb.reshape(CLS, 1))   # [CLS, 1]
    if v1:
        wT_full = np.ascontiguousarray(W.T).astype(BF)    # [N, N]

    in_maps = []
    for i in range(NCORES):
        xT = np.ascontiguousarray(x[i * BS : (i + 1) * BS].T).astype(BF)
        if v1:
            wT = wT_full
        else:
            wT = np.ascontiguousarray(W[i * WROWS : (i + 1) * WROWS].T).astype(BF)
        in_maps.append(
            {
                "xT": xT,
                "WT": wT,
                "fcwT": fcwT,
                "fc_b": fc_b,
                "lgwT": lgwT,
                "lgb": lgb_col,
            }
        )

    res = run_bass_kernel_spmd(
        nc,
        in_maps,
        core_ids=list(range(NCORES)),
        trace=bool(int(os.environ.get("KERNEL_TRACE", "0"))),
    )
    LAST_RESULT = res
    out = np.concatenate(
        [np.ascontiguousarray(r_["outT"].T) for r_ in res.results], axis=0
    )
    return out
